# revision 13
# baseline (speedup 1.0000x reference)
"""Trainium2 Bass kernel: PhonemeGRU (embedding -> GRU(scan over S) -> FC).

Strategy (per spec sharding hint): data-parallel over batch across 8 cores
(1024 rows/core). On each core the batch is partition-packed as 4 chunks x 32
features so every elementwise op runs on all 128 partitions. The embedding
lookup + input projection + all biases are folded into a 45-row table, applied
per step via one-hot matmuls (host precomputes one-hot in bf16, streamed to
SBUF); the hidden projection is 3 small (K=32,M=32) matmuls per chunk placed
at distinct PE tile positions so chunks run concurrently. Gate preacts land in
two PSUM banks laid out [R|Z] and [IN|HN] so sigmoid is a single (128,512)
ACT op and tanh a single (128,256) op per step. GRU state stays in bf16
(validated: 0.4% scale-relative absmax vs fp32 reference).
"""
import os
import sys

sys.path.insert(0, "/opt/trn_rl_repo")
sys.path.insert(0, "/opt/pypackages")
os.environ.setdefault("MYCRO_LOCAL_CACHE", "1")

from contextlib import ExitStack

import numpy as np
import ml_dtypes

import concourse.bass as bass
import concourse.bacc as bacc
import concourse.tile as tile
from concourse import mybir
from concourse.bass_utils import run_bass_kernel_spmd

BF16NP = ml_dtypes.bfloat16
BF16 = mybir.dt.bfloat16
F32 = mybir.dt.float32

VOCAB, E, H, O = 45, 16, 32, 64
B, S = 8192, 512
NCORES = 8
BC = B // NCORES          # 1024 rows per core
NCH = 4                   # partition-packed chunks
CW = BC // NCH            # 256 batch columns per chunk
TBLK = 16                 # steps per one-hot DMA block

Sigmoid = mybir.ActivationFunctionType.Sigmoid
Tanh = mybir.ActivationFunctionType.Tanh
Identity = mybir.ActivationFunctionType.Identity
ADD = mybir.AluOpType.add
MULT = mybir.AluOpType.mult


def _chunk_geo(c):
    """(h partition base, onehot partition base, onehot free offset) per chunk."""
    hp = 32 * c
    rp = 64 if c < 2 else 0   # chunks 0,1 read one-hot rows 64:109; 2,3 rows 0:45
    fo = CW if (c % 2) else 0
    return hp, rp, fo


def build_nc(seq_len=S):
    nc = bacc.Bacc("TRN2", target_bir_lowering=False, debug=False)

    oh_d = nc.dram_tensor("oh", [90, seq_len, 2 * CW], BF16, kind="ExternalInput").ap()
    wsb_d = nc.dram_tensor("wsb", [128, 96], BF16, kind="ExternalInput").ap()
    tsb_d = nc.dram_tensor("tsb", [128, 128], BF16, kind="ExternalInput").ap()
    fcw_d = nc.dram_tensor("fcw", [128, O], BF16, kind="ExternalInput").ap()
    fcb_d = nc.dram_tensor("fcb", [O, 1], F32, kind="ExternalInput").ap()
    out_d = nc.dram_tensor("out", [O, BC], F32, kind="ExternalOutput").ap()

    with tile.TileContext(nc) as tc:
        with ExitStack() as ctx:
            singles = ctx.enter_context(tc.tile_pool(name="singles", bufs=1))
            ohpool = ctx.enter_context(tc.tile_pool(name="ohpool", bufs=2))
            work = ctx.enter_context(tc.tile_pool(name="work", bufs=2))
            psum = ctx.enter_context(tc.tile_pool(name="psum", bufs=2, space="PSUM"))

            wsb = singles.tile([128, 96], BF16)
            tsb = singles.tile([128, 128], BF16)
            fcw = singles.tile([128, O], BF16)
            fcb = singles.tile([O, 1], F32)
            h = singles.tile([128, CW], BF16)

            nc.sync.dma_start(out=wsb, in_=wsb_d)
            nc.sync.dma_start(out=tsb, in_=tsb_d)
            nc.sync.dma_start(out=fcw, in_=fcw_d)
            nc.sync.dma_start(out=fcb, in_=fcb_d)
            nc.vector.memset(h, 0.0)

            tb = min(TBLK, seq_len)
            assert seq_len % tb == 0
            for blk in range(seq_len // tb):
                oh = ohpool.tile([128, tb, 2 * CW], BF16, tag="oh")
                t0 = blk * tb
                nc.sync.dma_start(out=oh[0:45], in_=oh_d[0:45, t0:t0 + tb])
                nc.sync.dma_start(out=oh[64:109], in_=oh_d[45:90, t0:t0 + tb])

                for j in range(tb):
                    pa = psum.tile([128, 2 * CW], F32, tag="pa")   # [R | Z]
                    pb = psum.tile([128, 2 * CW], F32, tag="pb")   # [IN | HN]
                    for c in range(NCH):
                        hp, rp, fo = _chunk_geo(c)
                        hc = h[hp:hp + 32, :]
                        ohc = oh[rp:rp + VOCAB, j, fo:fo + CW]
                        # R
                        nc.tensor.matmul(pa[hp:hp + 32, 0:CW], wsb[hp:hp + 32, 0:32],
                                         hc, start=True, stop=False,
                                         tile_position=(hp, hp))
                        nc.tensor.matmul(pa[hp:hp + 32, 0:CW], tsb[rp:rp + VOCAB, 0:32],
                                         ohc, start=False, stop=True,
                                         tile_position=(rp, hp))
                        # Z
                        nc.tensor.matmul(pa[hp:hp + 32, CW:2 * CW], wsb[hp:hp + 32, 32:64],
                                         hc, start=True, stop=False,
                                         tile_position=(hp, hp))
                        nc.tensor.matmul(pa[hp:hp + 32, CW:2 * CW], tsb[rp:rp + VOCAB, 32:64],
                                         ohc, start=False, stop=True,
                                         tile_position=(rp, hp))
                        # IN (table only)
                        nc.tensor.matmul(pb[hp:hp + 32, 0:CW], tsb[rp:rp + VOCAB, 64:96],
                                         ohc, start=True, stop=True,
                                         tile_position=(rp, hp))
                        # HN: hidden part + constant-row table block (= b_hh_n,
                        # since the one-hot column sums to 1)
                        nc.tensor.matmul(pb[hp:hp + 32, CW:2 * CW], wsb[hp:hp + 32, 64:96],
                                         hc, start=True, stop=False,
                                         tile_position=(hp, hp))
                        nc.tensor.matmul(pb[hp:hp + 32, CW:2 * CW], tsb[rp:rp + VOCAB, 96:128],
                                         ohc, start=False, stop=True,
                                         tile_position=(rp, hp))

                    rz = work.tile([128, 2 * CW], BF16, tag="rz")
                    nc.scalar.activation(rz, pa, Sigmoid)
                    tmp = work.tile([128, CW], BF16, tag="tmp")
                    # tmp = h_n * r
                    nc.vector.tensor_mul(tmp, pb[:, CW:2 * CW], rz[:, 0:CW])
                    # pre_n = i_n + tmp (in place in PSUM)
                    nc.vector.tensor_tensor(pb[:, 0:CW], pb[:, 0:CW], tmp, ADD)
                    n_t = work.tile([128, CW], BF16, tag="n")
                    nc.scalar.activation(n_t, pb[:, 0:CW], Tanh)
                    dlt = work.tile([128, CW], BF16, tag="dlt")
                    nc.vector.tensor_sub(dlt, h, n_t)
                    u = work.tile([128, CW], BF16, tag="u")
                    nc.gpsimd.tensor_tensor(u, rz[:, CW:2 * CW], dlt, MULT)
                    nc.vector.tensor_add(h, n_t, u)

            # FC head: out[o, b] = fc_w[o, :] @ h[b, :] + fc_b
            # Stage all chunks into one (32, BC) tile so the two N=512 matmuls
            # share row group 0 (serialized on PE; avoids the concurrent
            # same-partition PSUM-write hang probed in test_probe3 case B).
            h_all = singles.tile([32, NCH * CW], BF16)
            for c in range(NCH):
                nc.vector.tensor_copy(h_all[:, c * CW:(c + 1) * CW],
                                      h[32 * c:32 * c + 32, :])
            out_sb = singles.tile([O, NCH * CW], F32)
            half = NCH * CW // 2
            for k in range(2):
                pfc = psum.tile([O, half], F32, tag=f"pfc{k}")
                nc.tensor.matmul(pfc, fcw[0:32, :], h_all[:, k * half:(k + 1) * half],
                                 start=True, stop=True, tile_position=(0, 0))
                nc.scalar.activation(out_sb[:, k * half:(k + 1) * half], pfc,
                                     Identity, bias=fcb)
            nc.sync.dma_start(out=out_d, in_=out_sb)

    nc.compile()
    return nc


def host_prep(x, emb, w_ih, w_hh, b_ih, b_hh, fc_w, fc_b, seq_len=S):
    """Build per-core input maps (weights shared, one-hot per core)."""
    emb = np.asarray(emb, np.float32)
    w_ih = np.asarray(w_ih, np.float32)
    w_hh = np.asarray(w_hh, np.float32)
    b_ih = np.asarray(b_ih, np.float32)
    b_hh = np.asarray(b_hh, np.float32)
    fc_w = np.asarray(fc_w, np.float32)
    fc_b = np.asarray(fc_b, np.float32)
    x = np.asarray(x)

    table = emb @ w_ih.T  # (VOCAB, 3H)
    tfull = np.zeros((VOCAB, 128), np.float32)
    tfull[:, 0:32] = table[:, 0:32] + (b_ih + b_hh)[0:32]
    tfull[:, 32:64] = table[:, 32:64] + (b_ih + b_hh)[32:64]
    tfull[:, 64:96] = table[:, 64:96] + b_ih[64:96]
    tfull[:, 96:128] = b_hh[64:96][None, :]  # constant rows -> h_n bias

    wsb = np.zeros((128, 96), np.float32)
    for c in range(NCH):
        for g in range(3):
            wsb[32 * c:32 * c + 32, 32 * g:32 * g + 32] = w_hh[32 * g:32 * g + 32, :].T
    tsb = np.zeros((128, 128), np.float32)
    tsb[0:VOCAB] = tfull
    tsb[64:64 + VOCAB] = tfull
    fcw = np.zeros((128, O), np.float32)
    for c in range(NCH):
        fcw[32 * c:32 * c + 32, :] = fc_w.T
    fcb = fc_b[:, None].astype(np.float32)

    shared = {
        "wsb": wsb.astype(BF16NP),
        "tsb": tsb.astype(BF16NP),
        "fcw": fcw.astype(BF16NP),
        "fcb": fcb,
    }

    in_maps = []
    for core in range(NCORES):
        xc = x[core * BC:(core + 1) * BC, :seq_len]          # (BC, seq_len)
        oh = np.zeros((90, seq_len, 2 * CW), BF16NP)
        # chunks 0,1 -> dram rows 45:90 (sbuf 64:109); chunks 2,3 -> rows 0:45
        for c in range(NCH):
            _, rp, fo = _chunk_geo(c)
            base = 45 if rp == 64 else 0
            xcc = xc[c * CW:(c + 1) * CW].T                  # (seq_len, CW)
            for v in range(VOCAB):
                oh[base + v, :, fo:fo + CW] = (xcc == v)
        in_maps.append({"oh": oh, **shared})
    return in_maps


def assemble_output(results):
    outs = []
    for r in results:
        o = np.asarray(r["out"], np.float32)   # (O, BC) feature-major
        outs.append(o.T)                        # (BC, O)
    return np.concatenate(outs, axis=0)


def kernel(x, emb, w_ih, w_hh, b_ih, b_hh, fc_w, fc_b):
    nc = build_nc(S)
    in_maps = host_prep(x, emb, w_ih, w_hh, b_ih, b_hh, fc_w, fc_b, S)
    res = run_bass_kernel_spmd(nc, in_maps, core_ids=list(range(NCORES)))
    kernel._last_results = res
    return assemble_output(res.results)


kernel._last_results = None


# revision 19
# speedup vs baseline: 6.2928x; 6.2928x over previous
"""Trainium2 Bass kernel: PhonemeGRU (embedding -> GRU(scan over S) -> FC).

Strategy (per spec sharding hint): data-parallel over batch across 8 cores
(1024 rows/core). On each core the batch is partition-packed as 4 chunks x 32
features so every elementwise op runs on all 128 partitions. The embedding
lookup + input projection + all biases are folded into a 45-row table, applied
per step via one-hot matmuls (host precomputes one-hot in bf16, streamed to
SBUF); the hidden projection is 3 small (K=32,M=32) matmuls per chunk placed
at distinct PE tile positions so chunks run concurrently. Gate preacts land in
two PSUM banks laid out [R|Z] and [IN|HN] so sigmoid is a single (128,512)
ACT op and tanh a single (128,256) op per step. GRU state stays in bf16
(validated: 0.4% scale-relative absmax vs fp32 reference).
"""
import os
import sys

sys.path.insert(0, "/opt/trn_rl_repo")
sys.path.insert(0, "/opt/pypackages")
os.environ.setdefault("MYCRO_LOCAL_CACHE", "1")

from contextlib import ExitStack

import numpy as np
import ml_dtypes

import concourse.bass as bass
import concourse.bacc as bacc
import concourse.tile as tile
from concourse import mybir
from concourse.bass_utils import run_bass_kernel_spmd

BF16NP = ml_dtypes.bfloat16
BF16 = mybir.dt.bfloat16
F32 = mybir.dt.float32

VOCAB, E, H, O = 45, 16, 32, 64
B, S = 8192, 512
NCORES = 8
BC = B // NCORES          # 1024 rows per core
NCH = 4                   # partition-packed chunks
CW = BC // NCH            # 256 batch columns per chunk
TBLK = 16                 # steps per one-hot DMA block

Sigmoid = mybir.ActivationFunctionType.Sigmoid
Tanh = mybir.ActivationFunctionType.Tanh
Identity = mybir.ActivationFunctionType.Identity
ADD = mybir.AluOpType.add
MULT = mybir.AluOpType.mult


def _chunk_geo(c):
    """(h partition base, onehot partition base, onehot free offset) per chunk."""
    hp = 32 * c
    rp = 64 if c < 2 else 0   # chunks 0,1 read one-hot rows 64:109; 2,3 rows 0:45
    fo = CW if (c % 2) else 0
    return hp, rp, fo


def build_nc(seq_len=S):
    nc = bacc.Bacc("TRN2", target_bir_lowering=False, debug=False)

    oh_d = nc.dram_tensor("oh", [90, seq_len, 2 * CW], BF16, kind="ExternalInput").ap()
    wsb_d = nc.dram_tensor("wsb", [128, 96], BF16, kind="ExternalInput").ap()
    tsb_d = nc.dram_tensor("tsb", [128, 96], BF16, kind="ExternalInput").ap()
    bhat_d = nc.dram_tensor("bhat", [128, 1], F32, kind="ExternalInput").ap()
    fcw_d = nc.dram_tensor("fcw", [128, O], BF16, kind="ExternalInput").ap()
    fcb_d = nc.dram_tensor("fcb", [O, 1], F32, kind="ExternalInput").ap()
    out_d = nc.dram_tensor("out", [O, BC], F32, kind="ExternalOutput").ap()

    with tile.TileContext(nc) as tc:
        with ExitStack() as ctx:
            singles = ctx.enter_context(tc.tile_pool(name="singles", bufs=1))
            ohpool = ctx.enter_context(tc.tile_pool(name="ohpool", bufs=2))
            work = ctx.enter_context(tc.tile_pool(name="work", bufs=2))
            psum = ctx.enter_context(tc.tile_pool(name="psum", bufs=2, space="PSUM"))

            wsb = singles.tile([128, 96], BF16)
            tsb = singles.tile([128, 96], BF16)
            bhat = singles.tile([128, 1], F32)
            fcw = singles.tile([128, O], BF16)
            fcb = singles.tile([O, 1], F32)
            h = singles.tile([128, CW], BF16)

            nc.sync.dma_start(out=wsb, in_=wsb_d)
            nc.sync.dma_start(out=tsb, in_=tsb_d)
            nc.sync.dma_start(out=bhat, in_=bhat_d)
            nc.sync.dma_start(out=fcw, in_=fcw_d)
            nc.sync.dma_start(out=fcb, in_=fcb_d)
            nc.vector.memset(h, 0.0)

            tb = min(TBLK, seq_len)
            assert seq_len % tb == 0
            for blk in range(seq_len // tb):
                oh = ohpool.tile([128, tb, 2 * CW], BF16, tag="oh")
                t0 = blk * tb
                nc.sync.dma_start(out=oh[0:45], in_=oh_d[0:45, t0:t0 + tb])
                nc.sync.dma_start(out=oh[64:109], in_=oh_d[45:90, t0:t0 + tb])

                for j in range(tb):
                    pa = psum.tile([128, 2 * CW], F32, tag="pa")   # [R | Z]
                    pb = psum.tile([128, 2 * CW], F32, tag="pb")   # [IN | HN]
                    # One-hot (table) matmuls first with start=True: they only
                    # depend on the streamed one-hot block, so the PE can run
                    # them for step t+1 while step t's elementwise is in flight.
                    for c in range(NCH):
                        hp, rp, fo = _chunk_geo(c)
                        ohc = oh[rp:rp + VOCAB, j, fo:fo + CW]
                        nc.tensor.matmul(pa[hp:hp + 32, 0:CW], tsb[rp:rp + VOCAB, 0:32],
                                         ohc, start=True, stop=False,
                                         tile_position=(rp, hp))
                        nc.tensor.matmul(pa[hp:hp + 32, CW:2 * CW], tsb[rp:rp + VOCAB, 32:64],
                                         ohc, start=True, stop=False,
                                         tile_position=(rp, hp))
                        nc.tensor.matmul(pb[hp:hp + 32, 0:CW], tsb[rp:rp + VOCAB, 64:96],
                                         ohc, start=True, stop=True,
                                         tile_position=(rp, hp))
                    # Hidden-state matmuls accumulate on top (h-dependent).
                    for c in range(NCH):
                        hp, rp, fo = _chunk_geo(c)
                        hc = h[hp:hp + 32, :]
                        nc.tensor.matmul(pa[hp:hp + 32, 0:CW], wsb[hp:hp + 32, 0:32],
                                         hc, start=False, stop=True,
                                         tile_position=(hp, hp))
                        nc.tensor.matmul(pa[hp:hp + 32, CW:2 * CW], wsb[hp:hp + 32, 32:64],
                                         hc, start=False, stop=True,
                                         tile_position=(hp, hp))
                        nc.tensor.matmul(pb[hp:hp + 32, CW:2 * CW], wsb[hp:hp + 32, 64:96],
                                         hc, start=True, stop=True,
                                         tile_position=(hp, hp))

                    rz = work.tile([128, 2 * CW], BF16, tag="rz")
                    nc.scalar.activation(rz, pa, Sigmoid)
                    tmp = work.tile([128, CW], BF16, tag="tmp")
                    # tmp = (h_n_raw + b_hh_n) * r
                    nc.vector.scalar_tensor_tensor(tmp, pb[:, CW:2 * CW], bhat,
                                                   rz[:, 0:CW], ADD, MULT)
                    # pre_n = i_n + tmp (in place in PSUM)
                    nc.vector.tensor_tensor(pb[:, 0:CW], pb[:, 0:CW], tmp, ADD)
                    n_t = work.tile([128, CW], BF16, tag="n")
                    nc.scalar.activation(n_t, pb[:, 0:CW], Tanh)
                    dlt = work.tile([128, CW], BF16, tag="dlt")
                    nc.gpsimd.tensor_tensor(dlt, h, n_t, mybir.AluOpType.subtract)
                    u = work.tile([128, CW], BF16, tag="u")
                    nc.gpsimd.tensor_tensor(u, rz[:, CW:2 * CW], dlt, MULT)
                    nc.vector.tensor_add(h, n_t, u)

            # FC head: out[o, b] = fc_w[o, :] @ h[b, :] + fc_b
            # Stage all chunks into one (32, BC) tile so the two N=512 matmuls
            # share row group 0 (serialized on PE; avoids the concurrent
            # same-partition PSUM-write hang probed in test_probe3 case B).
            h_all = singles.tile([32, NCH * CW], BF16)
            for c in range(NCH):
                nc.vector.tensor_copy(h_all[:, c * CW:(c + 1) * CW],
                                      h[32 * c:32 * c + 32, :])
            out_sb = singles.tile([O, NCH * CW], F32)
            half = NCH * CW // 2
            for k in range(2):
                pfc = psum.tile([O, half], F32, tag=f"pfc{k}")
                nc.tensor.matmul(pfc, fcw[0:32, :], h_all[:, k * half:(k + 1) * half],
                                 start=True, stop=True, tile_position=(0, 0))
                nc.scalar.activation(out_sb[:, k * half:(k + 1) * half], pfc,
                                     Identity, bias=fcb)
            nc.sync.dma_start(out=out_d, in_=out_sb)

    nc.compile()
    return nc


def host_prep(x, emb, w_ih, w_hh, b_ih, b_hh, fc_w, fc_b, seq_len=S):
    """Build per-core input maps (weights shared, one-hot per core)."""
    emb = np.asarray(emb, np.float32)
    w_ih = np.asarray(w_ih, np.float32)
    w_hh = np.asarray(w_hh, np.float32)
    b_ih = np.asarray(b_ih, np.float32)
    b_hh = np.asarray(b_hh, np.float32)
    fc_w = np.asarray(fc_w, np.float32)
    fc_b = np.asarray(fc_b, np.float32)
    x = np.asarray(x)

    table = emb @ w_ih.T  # (VOCAB, 3H)
    tfull = np.zeros((VOCAB, 96), np.float32)
    tfull[:, 0:32] = table[:, 0:32] + (b_ih + b_hh)[0:32]
    tfull[:, 32:64] = table[:, 32:64] + (b_ih + b_hh)[32:64]
    tfull[:, 64:96] = table[:, 64:96] + b_ih[64:96]

    wsb = np.zeros((128, 96), np.float32)
    for c in range(NCH):
        for g in range(3):
            wsb[32 * c:32 * c + 32, 32 * g:32 * g + 32] = w_hh[32 * g:32 * g + 32, :].T
    tsb = np.zeros((128, 96), np.float32)
    tsb[0:VOCAB] = tfull
    tsb[64:64 + VOCAB] = tfull
    bhat = np.tile(b_hh[64:96], NCH)[:, None].astype(np.float32)
    fcw = np.zeros((128, O), np.float32)
    for c in range(NCH):
        fcw[32 * c:32 * c + 32, :] = fc_w.T
    fcb = fc_b[:, None].astype(np.float32)

    shared = {
        "wsb": wsb.astype(BF16NP),
        "tsb": tsb.astype(BF16NP),
        "bhat": bhat,
        "fcw": fcw.astype(BF16NP),
        "fcb": fcb,
    }

    in_maps = []
    for core in range(NCORES):
        xc = x[core * BC:(core + 1) * BC, :seq_len]          # (BC, seq_len)
        oh = np.zeros((90, seq_len, 2 * CW), BF16NP)
        # chunks 0,1 -> dram rows 45:90 (sbuf 64:109); chunks 2,3 -> rows 0:45
        for c in range(NCH):
            _, rp, fo = _chunk_geo(c)
            base = 45 if rp == 64 else 0
            xcc = xc[c * CW:(c + 1) * CW].T                  # (seq_len, CW)
            for v in range(VOCAB):
                oh[base + v, :, fo:fo + CW] = (xcc == v)
        in_maps.append({"oh": oh, **shared})
    return in_maps


def assemble_output(results):
    outs = []
    for r in results:
        o = np.asarray(r["out"], np.float32)   # (O, BC) feature-major
        outs.append(o.T)                        # (BC, O)
    return np.concatenate(outs, axis=0)


def kernel(x, emb, w_ih, w_hh, b_ih, b_hh, fc_w, fc_b):
    nc = build_nc(S)
    in_maps = host_prep(x, emb, w_ih, w_hh, b_ih, b_hh, fc_w, fc_b, S)
    res = run_bass_kernel_spmd(nc, in_maps, core_ids=list(range(NCORES)))
    kernel._last_results = res
    return assemble_output(res.results)


kernel._last_results = None
